# revision 14
# baseline (speedup 1.0000x reference)
"""DiffS6 (differential Mamba selective-scan block) TRN2 Bass kernel, v2.

d_inner sharded 8 ways (256 channels/core). Front (in_proj, conv,
x_proj, AllReduce) pipelined over two L-halves so the DVE scan loop
starts after the first half-collective. Scan-loop multiplies are
load-balanced between DVE and GpSimd (Pool; adds stay on DVE — Pool
adds are 8us). Branch-1 state sums accumulate on the PE via +/-identity
matmuls into PSUM, yielding y0-y1 without DVE adds. Early branch-0
iterations run split at the L/2 boundary with chained scan state.
"""
import numpy as np

NCORES = 8
D_MODEL = 1024
D_INNER = 2048
D_STATE = 16
D_CONV = 4
DT_RANK = 64
L = 2048
H = L // 2
DLOC = D_INNER // NCORES      # 256
NDT = DLOC // 128             # 2
P = 128
TC = 512
NTC = L // TC                 # 4
NKC = D_MODEL // P            # 8
SPLIT_N = 8                   # br0 n<SPLIT_N run split at the L/2 boundary

# measured per-op costs (ns)
DVE_TT = 1250
DVE_TT_H = 680
POOL_TT = 5100
POOL_TT_H = 2650

_CACHE = {}


class Planner:
    """Greedy DVE/Pool load balancer. Only multiplies go to Pool."""

    def __init__(self, nc):
        self.nc = nc
        self.load = {"v": 0.0, "p": 0.0}
        self.pool_ok = False   # no Pool ops before the last collective

    def mult(self, cols):
        dc = DVE_TT if cols == L else DVE_TT_H
        pc = POOL_TT if cols == L else POOL_TT_H
        if self.pool_ok and self.load["p"] + pc < self.load["v"] + dc:
            self.load["p"] += pc
            return self.nc.gpsimd
        self.load["v"] += dc
        return self.nc.vector

    def dve(self, cols, scan=False):
        if scan:
            self.load["v"] += 4450 if cols == L else 2280
        else:
            self.load["v"] += DVE_TT if cols == L else DVE_TT_H
        return self.nc.vector


def _build():
    from contextlib import ExitStack

    import concourse.mybir as mybir
    import concourse.tile as tile
    from concourse import bacc

    F32 = mybir.dt.float32
    F16 = mybir.dt.float16
    AT = mybir.ActivationFunctionType
    OP = mybir.AluOpType

    nc = bacc.Bacc("TRN2", target_bir_lowering=False, debug=False,
                   enable_asserts=False, num_devices=NCORES)

    hT_d = nc.dram_tensor("hT", [D_MODEL, L], F16, kind="ExternalInput")
    ipwT_d = nc.dram_tensor("ipwT", [D_MODEL, 2 * DLOC], F16, kind="ExternalInput")
    convw_d = nc.dram_tensor("convw", [DLOC, D_CONV], F32, kind="ExternalInput")
    convb_d = nc.dram_tensor("convb", [DLOC, 1], F32, kind="ExternalInput")
    xpwT_d = nc.dram_tensor("xpwT", [DLOC, 192], F16, kind="ExternalInput")
    dtpwT_d = nc.dram_tensor("dtpwT", [2, DT_RANK, DLOC], F32, kind="ExternalInput")
    dtb_d = nc.dram_tensor("dtb", [2, DLOC, 1], F32, kind="ExternalInput")
    acol_d = nc.dram_tensor("acol", [2, DLOC, D_STATE], F32, kind="ExternalInput")
    ddiff_d = nc.dram_tensor("ddiff", [DLOC, 1], F32, kind="ExternalInput")
    opwT_d = nc.dram_tensor("opwT", [DLOC, D_MODEL], F16, kind="ExternalInput")
    idm_d = nc.dram_tensor("idm", [2, P, P], F16, kind="ExternalInput")
    out_d = nc.dram_tensor("outp", [NDT, D_MODEL, L], F16, kind="ExternalOutput")

    dbldt_in = nc.dram_tensor("dbldt_in", [2, 2, DT_RANK, H], F32, kind="Internal")
    dbldt_out = nc.dram_tensor("dbldt_out", [2, 2, DT_RANK, H], F32,
                               kind="Internal", addr_space="Shared")
    dblbc_in = nc.dram_tensor("dblbc_in", [2, 2, 2 * D_STATE, H], F16, kind="Internal")
    dblbc_out = nc.dram_tensor("dblbc_out", [2, 2, 2 * D_STATE, H], F16,
                               kind="Internal", addr_space="Shared")

    pl = Planner(nc)

    with tile.TileContext(nc) as tc, ExitStack() as ex:
        wp = ex.enter_context(tc.tile_pool(name="wts", bufs=1))
        hp = ex.enter_context(tc.tile_pool(name="ht", bufs=3))
        bigp = ex.enter_context(tc.tile_pool(name="big", bufs=1))
        bcp = ex.enter_context(tc.tile_pool(name="bc", bufs=3))
        bchp = ex.enter_context(tc.tile_pool(name="bch", bufs=3))
        dap = ex.enter_context(tc.tile_pool(name="da", bufs=3))
        dahp = ex.enter_context(tc.tile_pool(name="dah", bufs=3))
        hpool = ex.enter_context(tc.tile_pool(name="h", bufs=3))
        hhp = ex.enter_context(tc.tile_pool(name="hh", bufs=3))
        gp = ex.enter_context(tc.tile_pool(name="g", bufs=3))
        ghp = ex.enter_context(tc.tile_pool(name="gh", bufs=3))
        yap = ex.enter_context(tc.tile_pool(name="ya", bufs=2))
        yahp = ex.enter_context(tc.tile_pool(name="yah", bufs=2))
        stp = ex.enter_context(tc.tile_pool(name="st", bufs=1))
        cvp = ex.enter_context(tc.tile_pool(name="conv", bufs=2))
        op_ = ex.enter_context(tc.tile_pool(name="osb", bufs=1))
        mmp2 = ex.enter_context(tc.tile_pool(name="mm2", bufs=1, space="PSUM"))
        mmp = ex.enter_context(tc.tile_pool(name="mm", bufs=2, space="PSUM"))
        ypp = ex.enter_context(tc.tile_pool(name="yps", bufs=1, space="PSUM"))

        # ---- weights ----
        ipwT = []
        for kc in range(NKC):
            t = wp.tile([P, 2 * DLOC], F16, tag=f"ipwT{kc}")
            nc.sync.dma_start(t[:], ipwT_d[kc * P:(kc + 1) * P, :])
            ipwT.append(t)
        xpwT = []
        for dt in range(NDT):
            t = wp.tile([P, 192], F16, tag=f"xpwT{dt}")
            nc.sync.dma_start(t[:], xpwT_d[dt * P:(dt + 1) * P, :])
            xpwT.append(t)
        dtpwT = []
        for br in range(2):
            t = wp.tile([DT_RANK, DLOC], F32, tag=f"dtpwT{br}")
            nc.sync.dma_start(t[:], dtpwT_d[br])
            dtpwT.append(t)
        opwT = []
        for dt in range(NDT):
            t = wp.tile([P, D_MODEL], F16, tag=f"opwT{dt}")
            nc.sync.dma_start(t[:], opwT_d[dt * P:(dt + 1) * P, :])
            opwT.append(t)
        idm = wp.tile([P, P], F16, tag="idm")
        nc.sync.dma_start(idm[:], idm_d[0])
        nidm = wp.tile([P, P], F16, tag="nidm")
        nc.sync.dma_start(nidm[:], idm_d[1])
        convw, convb, ddiff = [], [], []
        dtb, acol = {}, {}
        for dt in range(NDT):
            t = wp.tile([P, D_CONV], F32, tag=f"convw{dt}")
            nc.sync.dma_start(t[:], convw_d[dt * P:(dt + 1) * P, :])
            convw.append(t)
            t = wp.tile([P, 1], F32, tag=f"convb{dt}")
            nc.sync.dma_start(t[:], convb_d[dt * P:(dt + 1) * P, :])
            convb.append(t)
            t = wp.tile([P, 1], F32, tag=f"ddiff{dt}")
            nc.sync.dma_start(t[:], ddiff_d[dt * P:(dt + 1) * P, :])
            ddiff.append(t)
            for br in range(2):
                t = wp.tile([P, 1], F32, tag=f"dtb{br}{dt}")
                nc.sync.dma_start(t[:], dtb_d[br, dt * P:(dt + 1) * P, :])
                dtb[br, dt] = t
                t = wp.tile([P, D_STATE], F32, tag=f"acol{br}{dt}")
                nc.sync.dma_start(t[:], acol_d[br, dt * P:(dt + 1) * P, :])
                acol[br, dt] = t

        # ---- persistent activations ----
        x16 = [bigp.tile([P, L + 3], F16, tag=f"x16_{dt}", name=f"x16_{dt}")
               for dt in range(NDT)]
        z16 = [bigp.tile([P, L], F16, tag=f"z16_{dt}", name=f"z16_{dt}")
               for dt in range(NDT)]
        u16 = [bigp.tile([P, L], F16, tag=f"u16_{dt}", name=f"u16_{dt}")
               for dt in range(NDT)]
        delta = {(br, dt): bigp.tile([P, L], F16, tag=f"dl{br}{dt}",
                                     name=f"dl{br}{dt}")
                 for br in range(2) for dt in range(NDT)}
        v16 = {(br, dt): bigp.tile([P, L], F16, tag=f"v{br}{dt}",
                                   name=f"v{br}{dt}")
               for br in range(2) for dt in range(NDT)}
        y0 = [bigp.tile([P, L], F16, tag=f"y0_{dt}", name=f"y0_{dt}")
              for dt in range(NDT)]
        for dt in range(NDT):
            nc.vector.memset(x16[dt][:, 0:3], 0.0)

        def front_half(h, mmp):
            for tcc in (2 * h, 2 * h + 1):
                pss = [mmp.tile([P, TC], F32, tag="mm", name=f"ps{i}")
                       for i in range(2)]
                for kc in range(NKC):
                    ht = hp.tile([P, TC], F16, tag="ht")
                    nc.sync.dma_start(ht[:], hT_d[kc * P:(kc + 1) * P,
                                                  tcc * TC:(tcc + 1) * TC])
                    for rt in range(2):
                        nc.tensor.matmul(pss[rt][:],
                                         ipwT[kc][:, rt * P:(rt + 1) * P],
                                         ht[:], start=(kc == 0),
                                         stop=(kc == NKC - 1))
                for rt in range(2):
                    nc.scalar.copy(x16[rt][:, 3 + tcc * TC:3 + (tcc + 1) * TC],
                                   pss[rt][:])
                for dt in range(NDT):
                    cacc = cvp.tile([P, TC], F16, tag="conv")
                    s, e = tcc * TC, (tcc + 1) * TC
                    nc.vector.tensor_scalar(cacc[:], x16[dt][:, s:e],
                                            convw[dt][:, 0:1], None, OP.mult)
                    for k in range(1, D_CONV):
                        cacc2 = cvp.tile([P, TC], F16, tag="conv")
                        nc.vector.scalar_tensor_tensor(
                            cacc2[:], x16[dt][:, s + k:e + k],
                            convw[dt][:, k:k + 1],
                            cacc[:], OP.mult, OP.add)
                        cacc = cacc2
                    nc.scalar.activation(u16[dt][:, s:e], cacc[:], AT.Silu,
                                         bias=convb[dt][:, 0:1], scale=1.0)
            for br in range(2):
                for tcc in (2 * h, 2 * h + 1):
                    ps = mmp2.tile([96, TC], F32, tag="mm96")
                    for dt in range(NDT):
                        nc.tensor.matmul(ps[:],
                                         xpwT[dt][:, br * 96:(br + 1) * 96],
                                         u16[dt][:, tcc * TC:(tcc + 1) * TC],
                                         start=(dt == 0), stop=(dt == NDT - 1))
                    off = (tcc - 2 * h) * TC
                    evd = cvp.tile([DT_RANK, TC], F32, tag="dbl_ev", name="evd")
                    nc.scalar.copy(evd[:], ps[0:DT_RANK, :])
                    nc.sync.dma_start(dbldt_in[h, br, :, off:off + TC], evd[:])
                    evb = cvp.tile([2 * D_STATE, TC], F16, tag="ev_bc", name="evb")
                    nc.scalar.copy(evb[:], ps[DT_RANK:96, :])
                    nc.sync.dma_start(dblbc_in[h, br, :, off:off + TC], evb[:])

        def collectives(h):
            nc.gpsimd.collective_compute(
                "AllReduce", OP.add,
                replica_groups=[list(range(NCORES))],
                ins=[dblbc_in[h].opt()], outs=[dblbc_out[h].opt()])
            nc.gpsimd.collective_compute(
                "AllReduce", OP.add,
                replica_groups=[list(range(NCORES))],
                ins=[dbldt_in[h].opt()], outs=[dbldt_out[h].opt()])

        front_half(0, mmp)
        collectives(0)
        front_half(1, mmp)
        collectives(1)

        def prep_half(h, branches):
            hs = h * H
            for br in branches:
                dtr = cvp.tile([DT_RANK, H], F32, tag="dtr", name=f"dtr{br}{h}")
                nc.sync.dma_start(dtr[:], dbldt_out[h, br])
                for dt in range(NDT):
                    for c in range(H // TC):
                        ps = mmp2.tile([P, TC], F32, tag="mmdt", name="psd")
                        cs = hs + c * TC
                        nc.tensor.matmul(ps[:],
                                         dtpwT[br][:, dt * P:(dt + 1) * P],
                                         dtr[:, c * TC:(c + 1) * TC],
                                         start=True, stop=True)
                        # softplus(x) = ln(exp(x) + 1)
                        nc.scalar.activation(delta[br, dt][:, cs:cs + TC],
                                             ps[:], AT.Exp,
                                             bias=dtb[br, dt][:, 0:1], scale=1.0)
                        nc.scalar.activation(delta[br, dt][:, cs:cs + TC],
                                             delta[br, dt][:, cs:cs + TC],
                                             AT.Ln, bias=1.0)
                    pl.mult(H).tensor_tensor(v16[br, dt][:, hs:hs + H],
                                             delta[br, dt][:, hs:hs + H],
                                             u16[dt][:, hs:hs + H], OP.mult)

        def bcast_half(br, n, h):
            b = bchp.tile([P, H], F16, tag="bch", name="bch")
            nc.sync.dma_start(
                b[:], dblbc_out[h, br, n:n + 1, :].broadcast_to((P, H)))
            return b

        def bcast_full(br, n):
            b = bcp.tile([P, L], F16, tag="bc", name="bcf")
            for h in range(2):
                nc.sync.dma_start(
                    b[:, h * H:(h + 1) * H],
                    dblbc_out[h, br, n:n + 1, :].broadcast_to((P, H)))
            return b

        # ---------------- split iterations (br0, n < SPLIT_N), h0 --------
        yA, yC, yB = {}, {}, {}
        states = {}

        prep_half(0, [0])
        for n in range(SPLIT_N):
            bb = bcast_half(0, n, 0)
            cb = bcast_half(0, D_STATE + n, 0)
            for dt in range(NDT):
                dA = dahp.tile([P, H], F16, tag="dah")
                nc.scalar.activation(dA[:], delta[0, dt][:, 0:H], AT.Exp,
                                     bias=0.0, scale=acol[0, dt][:, n:n + 1])
                dBu = dahp.tile([P, H], F16, tag="dah")
                pl.dve(H).tensor_tensor(dBu[:], v16[0, dt][:, 0:H], bb[:],
                                        OP.mult)
                hh = hhp.tile([P, H], F16, tag="hh")
                pl.dve(H, scan=True).tensor_tensor_scan(
                    hh[:], dA[:], dBu[:], 0.0, OP.mult, OP.add)
                st = stp.tile([P, 1], F16, tag=f"st{n}_{dt}", name=f"st{n}_{dt}")
                nc.scalar.copy(st[:], hh[:, H - 1:H])
                states[n, dt] = st
                if n == 0:
                    ya = yahp.tile([P, H], F16, tag=f"yA{dt}", name="yA")
                    pl.dve(H).tensor_tensor(ya[:], hh[:], cb[:], OP.mult)
                else:
                    g = ghp.tile([P, H], F16, tag="gh")
                    pl.dve(H).tensor_tensor(g[:], hh[:], cb[:], OP.mult)
                    ya = yahp.tile([P, H], F16, tag=f"yA{dt}", name="yA")
                    pl.dve(H).tensor_tensor(ya[:], yA[dt][:], g[:], OP.add)
                yA[dt] = ya

        # z-rows in_proj (PE fills the wait for half-1 data)
        if True:
            for tcc in range(NTC):
                pss = [mmp.tile([P, TC], F32, tag="mm", name=f"psz{i}")
                       for i in range(2)]
                for kc in range(NKC):
                    ht = hp.tile([P, TC], F16, tag="ht")
                    nc.sync.dma_start(ht[:], hT_d[kc * P:(kc + 1) * P,
                                                  tcc * TC:(tcc + 1) * TC])
                    for rt in range(2):
                        nc.tensor.matmul(pss[rt][:],
                                         ipwT[kc][:, (rt + 2) * P:(rt + 3) * P],
                                         ht[:], start=(kc == 0),
                                         stop=(kc == NKC - 1))
                for rt in range(2):
                    nc.scalar.copy(z16[rt][:, tcc * TC:(tcc + 1) * TC],
                                   pss[rt][:])

            prep_half(0, [1])
            prep_half(1, [0, 1])
            pl.pool_ok = True

            # ------------- split iterations, h1 parts --------------------
            for n in range(SPLIT_N):
                bb = bcast_half(0, n, 1)
                cb = bcast_half(0, D_STATE + n, 1)
                for dt in range(NDT):
                    dA = dahp.tile([P, H], F16, tag="dah")
                    nc.scalar.activation(dA[:], delta[0, dt][:, H:L], AT.Exp,
                                         bias=0.0, scale=acol[0, dt][:, n:n + 1])
                    dBu = dahp.tile([P, H], F16, tag="dah")
                    pl.mult(H).tensor_tensor(dBu[:], v16[0, dt][:, H:L], bb[:],
                                             OP.mult)
                    hh = hhp.tile([P, H], F16, tag="hh")
                    pl.dve(H, scan=True).tensor_tensor_scan(
                        hh[:], dA[:], dBu[:], states[n, dt][:, 0:1],
                        OP.mult, OP.add)
                    if n == 0:
                        yc = yahp.tile([P, H], F16, tag=f"yC{dt}", name="yC")
                        pl.mult(H).tensor_tensor(yc[:], hh[:], cb[:], OP.mult)
                    else:
                        g = ghp.tile([P, H], F16, tag="gh")
                        pl.mult(H).tensor_tensor(g[:], hh[:], cb[:], OP.mult)
                        yc = yahp.tile([P, H], F16, tag=f"yC{dt}", name="yC")
                        pl.dve(H).tensor_tensor(yc[:], yC[dt][:], g[:], OP.add)
                    yC[dt] = yc

            # ------------- br0 full iterations (n-outer, dt-inner) -------
            for n in range(SPLIT_N, D_STATE):
                bb = bcast_full(0, n)
                cb = bcast_full(0, D_STATE + n)
                for dt in range(NDT):
                    dA = dap.tile([P, L], F16, tag="da")
                    nc.scalar.activation(dA[:], delta[0, dt][:], AT.Exp,
                                         bias=0.0, scale=acol[0, dt][:, n:n + 1])
                    dBu = dap.tile([P, L], F16, tag="da")
                    pl.mult(L).tensor_tensor(dBu[:], v16[0, dt][:], bb[:],
                                             OP.mult)
                    hh = hpool.tile([P, L], F16, tag="h")
                    pl.dve(L, scan=True).tensor_tensor_scan(
                        hh[:], dA[:], dBu[:], 0.0, OP.mult, OP.add)
                    if n == SPLIT_N:
                        yb = yap.tile([P, L], F16, tag=f"yB{dt}", name="yB")
                        pl.mult(L).tensor_tensor(yb[:], hh[:], cb[:], OP.mult)
                    else:
                        g = gp.tile([P, L], F16, tag="g")
                        pl.mult(L).tensor_tensor(g[:], hh[:], cb[:], OP.mult)
                        yb = yap.tile([P, L], F16, tag=f"yB{dt}", name="yB")
                        pl.dve(L).tensor_tensor(yb[:], yB[dt][:], g[:], OP.add)
                    yB[dt] = yb

            # combine br0 partial sums into y0
            for dt in range(NDT):
                pl.dve(H).tensor_tensor(y0[dt][:, 0:H], yB[dt][:, 0:H],
                                        yA[dt][:], OP.add)
                pl.dve(H).tensor_tensor(y0[dt][:, H:L], yB[dt][:, H:L],
                                        yC[dt][:], OP.add)

            # ------------- branch 1: dt-outer, PE psum accumulation ------
            pending = []

            def emit_out(dt, ygt, ot):
                osb = op_.tile([P, L], F16, tag="osb", name="osb")
                for tcc in range(NTC):
                    ps = mmp.tile([P, TC], F32, tag="mm", name="pso")
                    nc.tensor.matmul(ps[:], opwT[dt][:, ot * P:(ot + 1) * P],
                                     ygt[:, tcc * TC:(tcc + 1) * TC],
                                     start=True, stop=True)
                    nc.scalar.copy(osb[:, tcc * TC:(tcc + 1) * TC], ps[:])
                nc.sync.dma_start(out_d[dt, ot * P:(ot + 1) * P, :], osb[:])

            if True:
                for dt in range(NDT):
                    yps = ypp.tile([P, L], F32, tag="yps", name=f"yps{dt}")
                    # seed with +y0
                    for c in range(NTC):
                        nc.tensor.matmul(yps[:, c * TC:(c + 1) * TC], idm[:],
                                         y0[dt][:, c * TC:(c + 1) * TC],
                                         start=True, stop=False)
                    for n in range(D_STATE):
                        bb = bcast_full(1, n)
                        cb = bcast_full(1, D_STATE + n)
                        dA = dap.tile([P, L], F16, tag="da")
                        nc.scalar.activation(dA[:], delta[1, dt][:], AT.Exp,
                                             bias=0.0,
                                             scale=acol[1, dt][:, n:n + 1])
                        dBu = dap.tile([P, L], F16, tag="da")
                        pl.mult(L).tensor_tensor(dBu[:], v16[1, dt][:], bb[:],
                                                 OP.mult)
                        hh = hpool.tile([P, L], F16, tag="h")
                        pl.dve(L, scan=True).tensor_tensor_scan(
                            hh[:], dA[:], dBu[:], 0.0, OP.mult, OP.add)
                        g = gp.tile([P, L], F16, tag="g")
                        pl.mult(L).tensor_tensor(g[:], hh[:], cb[:], OP.mult)
                        # accumulate -g into psum
                        for c in range(NTC):
                            nc.tensor.matmul(yps[:, c * TC:(c + 1) * TC],
                                             nidm[:],
                                             g[:, c * TC:(c + 1) * TC],
                                             start=False,
                                             stop=(n == D_STATE - 1))
                        if pending and 4 <= n <= 11:
                            pdt, pygt = pending[0]
                            emit_out(pdt, pygt, n - 4)
                            if n == 11:
                                pending.pop(0)

                    # ydiff(+D-term) from psum, then gate
                    yd2 = gp.tile([P, L], F16, tag="g")
                    pl.dve(L)
                    nc.vector.scalar_tensor_tensor(yd2[:], u16[dt][:],
                                                   ddiff[dt][:, 0:1], yps[:],
                                                   OP.mult, OP.add)
                    sz = gp.tile([P, L], F16, tag="g")
                    nc.scalar.activation(sz[:], z16[dt][:], AT.Silu)
                    ygt = bigp.tile([P, L + 3], F16, tag=f"x16_{dt}")
                    pl.mult(L).tensor_tensor(ygt[:, 0:L], yd2[:], sz[:],
                                             OP.mult)
                    if dt == NDT - 1:
                        for ot in range(D_MODEL // P):
                            emit_out(dt, ygt, ot)
                    else:
                        pending.append((dt, ygt))

    nc.finalize()
    return nc


def _get_nc():
    if "nc" not in _CACHE:
        _CACHE["nc"] = _build()
    return _CACHE["nc"]


def kernel(hidden_states, in_proj_w, conv_w, conv_b,
           x1_proj_w, dt1_proj_w, dt1_proj_b, A1_log, D1,
           x2_proj_w, dt2_proj_w, dt2_proj_b, A2_log, D2,
           out_proj_w):
    import os
    from concourse.bass_utils import run_bass_kernel_spmd
    try:
        import antenv.axon_hooks  # noqa: F401
    except ImportError:
        os.environ["BASS_NEVER_TRACE"] = "1"

    f32 = np.float32
    f16 = np.float16
    hidden_states = np.asarray(hidden_states, f32)
    in_proj_w = np.asarray(in_proj_w, f32)
    conv_w = np.asarray(conv_w, f32)
    conv_b = np.asarray(conv_b, f32)
    out_proj_w = np.asarray(out_proj_w, f32)

    hT16 = np.ascontiguousarray(hidden_states[0].T).astype(f16)
    A1 = -np.exp(np.asarray(A1_log, f32))
    A2 = -np.exp(np.asarray(A2_log, f32))
    Dd = (np.asarray(D1, f32) - np.asarray(D2, f32))

    eye = np.eye(P, dtype=f16)
    idm = np.stack([eye, -eye])

    xp = [np.asarray(x1_proj_w, f32), np.asarray(x2_proj_w, f32)]
    dtpw = [np.asarray(dt1_proj_w, f32), np.asarray(dt2_proj_w, f32)]
    dtb = [np.asarray(dt1_proj_b, f32), np.asarray(dt2_proj_b, f32)]
    Acols = [A1, A2]

    in_maps = []
    for c in range(NCORES):
        ds = slice(c * DLOC, (c + 1) * DLOC)
        ipw_loc = np.concatenate([in_proj_w[ds], in_proj_w[D_INNER:][ds]], 0)
        in_maps.append({
            "hT": hT16,
            "ipwT": np.ascontiguousarray(ipw_loc.T).astype(f16),
            "convw": np.ascontiguousarray(conv_w[ds]).astype(f32),
            "convb": np.ascontiguousarray(conv_b[ds][:, None]).astype(f32),
            "xpwT": np.ascontiguousarray(
                np.concatenate([xp[0][:, ds], xp[1][:, ds]], 0).T).astype(f16),
            "dtpwT": np.ascontiguousarray(
                np.stack([dtpw[0][ds].T, dtpw[1][ds].T])).astype(f32),
            "dtb": np.ascontiguousarray(
                np.stack([dtb[0][ds][:, None], dtb[1][ds][:, None]])).astype(f32),
            "acol": np.ascontiguousarray(
                np.stack([Acols[0][ds], Acols[1][ds]])).astype(f32),
            "ddiff": np.ascontiguousarray(Dd[ds][:, None]).astype(f32),
            "opwT": np.ascontiguousarray(out_proj_w[:, ds].T).astype(f16),
            "idm": idm,
        })

    nc = _get_nc()
    res = run_bass_kernel_spmd(nc, in_maps, core_ids=list(range(NCORES)))
    _CACHE["last_res"] = res
    out = np.zeros((D_MODEL, L), f32)
    for r in res.results:
        out += r["outp"].astype(f32).sum(axis=0)
    return np.ascontiguousarray(out.T)[None].astype(f32)


# revision 15
# speedup vs baseline: 1.1436x; 1.1436x over previous
"""DiffS6 (differential Mamba selective-scan block) TRN2 Bass kernel, v2.

d_inner sharded 8 ways (256 channels/core). Front (in_proj, conv,
x_proj, AllReduce) pipelined over two L-halves so the DVE scan loop
starts after the first half-collective. Scan-loop multiplies are
load-balanced between DVE and GpSimd (Pool; adds stay on DVE — Pool
adds are 8us). Branch-1 state sums accumulate on the PE via +/-identity
matmuls into PSUM, yielding y0-y1 without DVE adds. Early branch-0
iterations run split at the L/2 boundary with chained scan state.
"""
import numpy as np

NCORES = 8
D_MODEL = 1024
D_INNER = 2048
D_STATE = 16
D_CONV = 4
DT_RANK = 64
L = 2048
H = L // 2
DLOC = D_INNER // NCORES      # 256
NDT = DLOC // 128             # 2
P = 128
TC = 512
NTC = L // TC                 # 4
NKC = D_MODEL // P            # 8
SPLIT_N = 8                   # br0 n<SPLIT_N run split at the L/2 boundary

# measured per-op costs (ns)
DVE_TT = 1250
DVE_TT_H = 680
POOL_TT = 5100
POOL_TT_H = 2650

_CACHE = {}


class Planner:
    """Greedy DVE/Pool load balancer. Only multiplies go to Pool."""

    def __init__(self, nc):
        self.nc = nc
        self.load = {"v": 0.0, "p": 0.0}
        self.pool_ok = False   # no Pool ops before the last collective

    def mult(self, cols):
        dc = DVE_TT if cols == L else DVE_TT_H
        pc = POOL_TT if cols == L else POOL_TT_H
        if False and self.pool_ok and self.load["p"] + pc < self.load["v"] + dc:
            self.load["p"] += pc
            return self.nc.gpsimd
        self.load["v"] += dc
        return self.nc.vector

    def dve(self, cols, scan=False):
        if scan:
            self.load["v"] += 4450 if cols == L else 2280
        else:
            self.load["v"] += DVE_TT if cols == L else DVE_TT_H
        return self.nc.vector


def _build():
    from contextlib import ExitStack

    import concourse.mybir as mybir
    import concourse.tile as tile
    from concourse import bacc

    F32 = mybir.dt.float32
    F16 = mybir.dt.float16
    AT = mybir.ActivationFunctionType
    OP = mybir.AluOpType

    nc = bacc.Bacc("TRN2", target_bir_lowering=False, debug=False,
                   enable_asserts=False, num_devices=NCORES)

    hT_d = nc.dram_tensor("hT", [D_MODEL, L], F16, kind="ExternalInput")
    ipwT_d = nc.dram_tensor("ipwT", [D_MODEL, 2 * DLOC], F16, kind="ExternalInput")
    convw_d = nc.dram_tensor("convw", [DLOC, D_CONV], F32, kind="ExternalInput")
    convb_d = nc.dram_tensor("convb", [DLOC, 1], F32, kind="ExternalInput")
    xpwT_d = nc.dram_tensor("xpwT", [DLOC, 192], F16, kind="ExternalInput")
    dtpwT_d = nc.dram_tensor("dtpwT", [2, DT_RANK, DLOC], F32, kind="ExternalInput")
    dtb_d = nc.dram_tensor("dtb", [2, DLOC, 1], F32, kind="ExternalInput")
    acol_d = nc.dram_tensor("acol", [2, DLOC, D_STATE], F32, kind="ExternalInput")
    ddiff_d = nc.dram_tensor("ddiff", [DLOC, 1], F32, kind="ExternalInput")
    opwT_d = nc.dram_tensor("opwT", [DLOC, D_MODEL], F16, kind="ExternalInput")
    idm_d = nc.dram_tensor("idm", [2, P, P], F16, kind="ExternalInput")
    out_d = nc.dram_tensor("outp", [NDT, D_MODEL, L], F16, kind="ExternalOutput")

    dbldt_in = nc.dram_tensor("dbldt_in", [2, 2, DT_RANK, H], F32, kind="Internal")
    dbldt_out = nc.dram_tensor("dbldt_out", [2, 2, DT_RANK, H], F32,
                               kind="Internal", addr_space="Shared")
    dblbc_in = nc.dram_tensor("dblbc_in", [2, 2, 2 * D_STATE, H], F16, kind="Internal")
    dblbc_out = nc.dram_tensor("dblbc_out", [2, 2, 2 * D_STATE, H], F16,
                               kind="Internal", addr_space="Shared")

    pl = Planner(nc)

    with tile.TileContext(nc) as tc, ExitStack() as ex:
        wp = ex.enter_context(tc.tile_pool(name="wts", bufs=1))
        hp = ex.enter_context(tc.tile_pool(name="ht", bufs=3))
        bigp = ex.enter_context(tc.tile_pool(name="big", bufs=1))
        bcp = ex.enter_context(tc.tile_pool(name="bc", bufs=3))
        bchp = ex.enter_context(tc.tile_pool(name="bch", bufs=3))
        dap = ex.enter_context(tc.tile_pool(name="da", bufs=3))
        dahp = ex.enter_context(tc.tile_pool(name="dah", bufs=3))
        hpool = ex.enter_context(tc.tile_pool(name="h", bufs=3))
        hhp = ex.enter_context(tc.tile_pool(name="hh", bufs=3))
        gp = ex.enter_context(tc.tile_pool(name="g", bufs=3))
        ghp = ex.enter_context(tc.tile_pool(name="gh", bufs=3))
        yap = ex.enter_context(tc.tile_pool(name="ya", bufs=2))
        yahp = ex.enter_context(tc.tile_pool(name="yah", bufs=2))
        stp = ex.enter_context(tc.tile_pool(name="st", bufs=1))
        cvp = ex.enter_context(tc.tile_pool(name="conv", bufs=2))
        op_ = ex.enter_context(tc.tile_pool(name="osb", bufs=1))
        mmp2 = ex.enter_context(tc.tile_pool(name="mm2", bufs=1, space="PSUM"))
        mmp = ex.enter_context(tc.tile_pool(name="mm", bufs=2, space="PSUM"))
        ypp = ex.enter_context(tc.tile_pool(name="yps", bufs=1, space="PSUM"))

        # ---- weights ----
        ipwT = []
        for kc in range(NKC):
            t = wp.tile([P, 2 * DLOC], F16, tag=f"ipwT{kc}")
            nc.sync.dma_start(t[:], ipwT_d[kc * P:(kc + 1) * P, :])
            ipwT.append(t)
        xpwT = []
        for dt in range(NDT):
            t = wp.tile([P, 192], F16, tag=f"xpwT{dt}")
            nc.sync.dma_start(t[:], xpwT_d[dt * P:(dt + 1) * P, :])
            xpwT.append(t)
        dtpwT = []
        for br in range(2):
            t = wp.tile([DT_RANK, DLOC], F32, tag=f"dtpwT{br}")
            nc.sync.dma_start(t[:], dtpwT_d[br])
            dtpwT.append(t)
        opwT = []
        for dt in range(NDT):
            t = wp.tile([P, D_MODEL], F16, tag=f"opwT{dt}")
            nc.sync.dma_start(t[:], opwT_d[dt * P:(dt + 1) * P, :])
            opwT.append(t)
        idm = wp.tile([P, P], F16, tag="idm")
        nc.sync.dma_start(idm[:], idm_d[0])
        nidm = wp.tile([P, P], F16, tag="nidm")
        nc.sync.dma_start(nidm[:], idm_d[1])
        convw, convb, ddiff = [], [], []
        dtb, acol = {}, {}
        for dt in range(NDT):
            t = wp.tile([P, D_CONV], F32, tag=f"convw{dt}")
            nc.sync.dma_start(t[:], convw_d[dt * P:(dt + 1) * P, :])
            convw.append(t)
            t = wp.tile([P, 1], F32, tag=f"convb{dt}")
            nc.sync.dma_start(t[:], convb_d[dt * P:(dt + 1) * P, :])
            convb.append(t)
            t = wp.tile([P, 1], F32, tag=f"ddiff{dt}")
            nc.sync.dma_start(t[:], ddiff_d[dt * P:(dt + 1) * P, :])
            ddiff.append(t)
            for br in range(2):
                t = wp.tile([P, 1], F32, tag=f"dtb{br}{dt}")
                nc.sync.dma_start(t[:], dtb_d[br, dt * P:(dt + 1) * P, :])
                dtb[br, dt] = t
                t = wp.tile([P, D_STATE], F32, tag=f"acol{br}{dt}")
                nc.sync.dma_start(t[:], acol_d[br, dt * P:(dt + 1) * P, :])
                acol[br, dt] = t

        # ---- persistent activations ----
        x16 = [bigp.tile([P, L + 3], F16, tag=f"x16_{dt}", name=f"x16_{dt}")
               for dt in range(NDT)]
        z16 = [bigp.tile([P, L], F16, tag=f"z16_{dt}", name=f"z16_{dt}")
               for dt in range(NDT)]
        u16 = [bigp.tile([P, L], F16, tag=f"u16_{dt}", name=f"u16_{dt}")
               for dt in range(NDT)]
        delta = {(br, dt): bigp.tile([P, L], F16, tag=f"dl{br}{dt}",
                                     name=f"dl{br}{dt}")
                 for br in range(2) for dt in range(NDT)}
        v16 = {(br, dt): bigp.tile([P, L], F16, tag=f"v{br}{dt}",
                                   name=f"v{br}{dt}")
               for br in range(2) for dt in range(NDT)}
        y0 = [bigp.tile([P, L], F16, tag=f"y0_{dt}", name=f"y0_{dt}")
              for dt in range(NDT)]
        for dt in range(NDT):
            nc.vector.memset(x16[dt][:, 0:3], 0.0)

        def front_half(h, mmp):
            for tcc in (2 * h, 2 * h + 1):
                pss = [mmp.tile([P, TC], F32, tag="mm", name=f"ps{i}")
                       for i in range(2)]
                for kc in range(NKC):
                    ht = hp.tile([P, TC], F16, tag="ht")
                    nc.sync.dma_start(ht[:], hT_d[kc * P:(kc + 1) * P,
                                                  tcc * TC:(tcc + 1) * TC])
                    for rt in range(2):
                        nc.tensor.matmul(pss[rt][:],
                                         ipwT[kc][:, rt * P:(rt + 1) * P],
                                         ht[:], start=(kc == 0),
                                         stop=(kc == NKC - 1))
                for rt in range(2):
                    nc.scalar.copy(x16[rt][:, 3 + tcc * TC:3 + (tcc + 1) * TC],
                                   pss[rt][:])
                for dt in range(NDT):
                    cacc = cvp.tile([P, TC], F16, tag="conv")
                    s, e = tcc * TC, (tcc + 1) * TC
                    nc.vector.tensor_scalar(cacc[:], x16[dt][:, s:e],
                                            convw[dt][:, 0:1], None, OP.mult)
                    for k in range(1, D_CONV):
                        cacc2 = cvp.tile([P, TC], F16, tag="conv")
                        nc.vector.scalar_tensor_tensor(
                            cacc2[:], x16[dt][:, s + k:e + k],
                            convw[dt][:, k:k + 1],
                            cacc[:], OP.mult, OP.add)
                        cacc = cacc2
                    nc.scalar.activation(u16[dt][:, s:e], cacc[:], AT.Silu,
                                         bias=convb[dt][:, 0:1], scale=1.0)
            for br in range(2):
                for tcc in (2 * h, 2 * h + 1):
                    ps = mmp2.tile([96, TC], F32, tag="mm96")
                    for dt in range(NDT):
                        nc.tensor.matmul(ps[:],
                                         xpwT[dt][:, br * 96:(br + 1) * 96],
                                         u16[dt][:, tcc * TC:(tcc + 1) * TC],
                                         start=(dt == 0), stop=(dt == NDT - 1))
                    off = (tcc - 2 * h) * TC
                    evd = cvp.tile([DT_RANK, TC], F32, tag="dbl_ev", name="evd")
                    nc.scalar.copy(evd[:], ps[0:DT_RANK, :])
                    nc.sync.dma_start(dbldt_in[h, br, :, off:off + TC], evd[:])
                    evb = cvp.tile([2 * D_STATE, TC], F16, tag="ev_bc", name="evb")
                    nc.scalar.copy(evb[:], ps[DT_RANK:96, :])
                    nc.sync.dma_start(dblbc_in[h, br, :, off:off + TC], evb[:])

        def collectives(h):
            nc.gpsimd.collective_compute(
                "AllReduce", OP.add,
                replica_groups=[list(range(NCORES))],
                ins=[dblbc_in[h].opt()], outs=[dblbc_out[h].opt()])
            nc.gpsimd.collective_compute(
                "AllReduce", OP.add,
                replica_groups=[list(range(NCORES))],
                ins=[dbldt_in[h].opt()], outs=[dbldt_out[h].opt()])

        front_half(0, mmp)
        collectives(0)
        front_half(1, mmp)
        collectives(1)

        def prep_half(h, branches):
            hs = h * H
            for br in branches:
                dtr = cvp.tile([DT_RANK, H], F32, tag="dtr", name=f"dtr{br}{h}")
                nc.sync.dma_start(dtr[:], dbldt_out[h, br])
                for dt in range(NDT):
                    for c in range(H // TC):
                        ps = mmp2.tile([P, TC], F32, tag="mmdt", name="psd")
                        cs = hs + c * TC
                        nc.tensor.matmul(ps[:],
                                         dtpwT[br][:, dt * P:(dt + 1) * P],
                                         dtr[:, c * TC:(c + 1) * TC],
                                         start=True, stop=True)
                        # softplus(x) = ln(exp(x) + 1)
                        nc.scalar.activation(delta[br, dt][:, cs:cs + TC],
                                             ps[:], AT.Exp,
                                             bias=dtb[br, dt][:, 0:1], scale=1.0)
                        nc.scalar.activation(delta[br, dt][:, cs:cs + TC],
                                             delta[br, dt][:, cs:cs + TC],
                                             AT.Ln, bias=1.0)
                    pl.mult(H).tensor_tensor(v16[br, dt][:, hs:hs + H],
                                             delta[br, dt][:, hs:hs + H],
                                             u16[dt][:, hs:hs + H], OP.mult)

        def bcast_half(br, n, h):
            b = bchp.tile([P, H], F16, tag="bch", name="bch")
            nc.sync.dma_start(
                b[:], dblbc_out[h, br, n:n + 1, :].broadcast_to((P, H)))
            return b

        def bcast_full(br, n):
            b = bcp.tile([P, L], F16, tag="bc", name="bcf")
            for h in range(2):
                nc.sync.dma_start(
                    b[:, h * H:(h + 1) * H],
                    dblbc_out[h, br, n:n + 1, :].broadcast_to((P, H)))
            return b

        # ---------------- split iterations (br0, n < SPLIT_N), h0 --------
        yA, yC, yB = {}, {}, {}
        states = {}

        prep_half(0, [0])
        for n in range(SPLIT_N):
            bb = bcast_half(0, n, 0)
            cb = bcast_half(0, D_STATE + n, 0)
            for dt in range(NDT):
                dA = dahp.tile([P, H], F16, tag="dah")
                nc.scalar.activation(dA[:], delta[0, dt][:, 0:H], AT.Exp,
                                     bias=0.0, scale=acol[0, dt][:, n:n + 1])
                dBu = dahp.tile([P, H], F16, tag="dah")
                pl.dve(H).tensor_tensor(dBu[:], v16[0, dt][:, 0:H], bb[:],
                                        OP.mult)
                hh = hhp.tile([P, H], F16, tag="hh")
                pl.dve(H, scan=True).tensor_tensor_scan(
                    hh[:], dA[:], dBu[:], 0.0, OP.mult, OP.add)
                st = stp.tile([P, 1], F16, tag=f"st{n}_{dt}", name=f"st{n}_{dt}")
                nc.scalar.copy(st[:], hh[:, H - 1:H])
                states[n, dt] = st
                if n == 0:
                    ya = yahp.tile([P, H], F16, tag=f"yA{dt}", name="yA")
                    pl.dve(H).tensor_tensor(ya[:], hh[:], cb[:], OP.mult)
                else:
                    g = ghp.tile([P, H], F16, tag="gh")
                    pl.dve(H).tensor_tensor(g[:], hh[:], cb[:], OP.mult)
                    ya = yahp.tile([P, H], F16, tag=f"yA{dt}", name="yA")
                    pl.dve(H).tensor_tensor(ya[:], yA[dt][:], g[:], OP.add)
                yA[dt] = ya

        # z-rows in_proj (PE fills the wait for half-1 data)
        if True:
            for tcc in range(NTC):
                pss = [mmp.tile([P, TC], F32, tag="mm", name=f"psz{i}")
                       for i in range(2)]
                for kc in range(NKC):
                    ht = hp.tile([P, TC], F16, tag="ht")
                    nc.sync.dma_start(ht[:], hT_d[kc * P:(kc + 1) * P,
                                                  tcc * TC:(tcc + 1) * TC])
                    for rt in range(2):
                        nc.tensor.matmul(pss[rt][:],
                                         ipwT[kc][:, (rt + 2) * P:(rt + 3) * P],
                                         ht[:], start=(kc == 0),
                                         stop=(kc == NKC - 1))
                for rt in range(2):
                    nc.scalar.copy(z16[rt][:, tcc * TC:(tcc + 1) * TC],
                                   pss[rt][:])

            prep_half(0, [1])
            prep_half(1, [0, 1])
            pl.pool_ok = True

            # ------------- split iterations, h1 parts --------------------
            for n in range(SPLIT_N):
                bb = bcast_half(0, n, 1)
                cb = bcast_half(0, D_STATE + n, 1)
                for dt in range(NDT):
                    dA = dahp.tile([P, H], F16, tag="dah")
                    nc.scalar.activation(dA[:], delta[0, dt][:, H:L], AT.Exp,
                                         bias=0.0, scale=acol[0, dt][:, n:n + 1])
                    dBu = dahp.tile([P, H], F16, tag="dah")
                    pl.mult(H).tensor_tensor(dBu[:], v16[0, dt][:, H:L], bb[:],
                                             OP.mult)
                    hh = hhp.tile([P, H], F16, tag="hh")
                    pl.dve(H, scan=True).tensor_tensor_scan(
                        hh[:], dA[:], dBu[:], states[n, dt][:, 0:1],
                        OP.mult, OP.add)
                    if n == 0:
                        yc = yahp.tile([P, H], F16, tag=f"yC{dt}", name="yC")
                        pl.mult(H).tensor_tensor(yc[:], hh[:], cb[:], OP.mult)
                    else:
                        g = ghp.tile([P, H], F16, tag="gh")
                        pl.mult(H).tensor_tensor(g[:], hh[:], cb[:], OP.mult)
                        yc = yahp.tile([P, H], F16, tag=f"yC{dt}", name="yC")
                        pl.dve(H).tensor_tensor(yc[:], yC[dt][:], g[:], OP.add)
                    yC[dt] = yc

            # ------------- br0 full iterations (n-outer, dt-inner) -------
            for n in range(SPLIT_N, D_STATE):
                bb = bcast_full(0, n)
                cb = bcast_full(0, D_STATE + n)
                for dt in range(NDT):
                    dA = dap.tile([P, L], F16, tag="da")
                    nc.scalar.activation(dA[:], delta[0, dt][:], AT.Exp,
                                         bias=0.0, scale=acol[0, dt][:, n:n + 1])
                    dBu = dap.tile([P, L], F16, tag="da")
                    pl.mult(L).tensor_tensor(dBu[:], v16[0, dt][:], bb[:],
                                             OP.mult)
                    hh = hpool.tile([P, L], F16, tag="h")
                    pl.dve(L, scan=True).tensor_tensor_scan(
                        hh[:], dA[:], dBu[:], 0.0, OP.mult, OP.add)
                    if n == SPLIT_N:
                        yb = yap.tile([P, L], F16, tag=f"yB{dt}", name="yB")
                        pl.mult(L).tensor_tensor(yb[:], hh[:], cb[:], OP.mult)
                    else:
                        g = gp.tile([P, L], F16, tag="g")
                        pl.mult(L).tensor_tensor(g[:], hh[:], cb[:], OP.mult)
                        yb = yap.tile([P, L], F16, tag=f"yB{dt}", name="yB")
                        pl.dve(L).tensor_tensor(yb[:], yB[dt][:], g[:], OP.add)
                    yB[dt] = yb

            # combine br0 partial sums into y0
            for dt in range(NDT):
                pl.dve(H).tensor_tensor(y0[dt][:, 0:H], yB[dt][:, 0:H],
                                        yA[dt][:], OP.add)
                pl.dve(H).tensor_tensor(y0[dt][:, H:L], yB[dt][:, H:L],
                                        yC[dt][:], OP.add)

            # ------------- branch 1: dt-outer, PE psum accumulation ------
            pending = []

            def emit_out(dt, ygt, ot):
                osb = op_.tile([P, L], F16, tag="osb", name="osb")
                for tcc in range(NTC):
                    ps = mmp.tile([P, TC], F32, tag="mm", name="pso")
                    nc.tensor.matmul(ps[:], opwT[dt][:, ot * P:(ot + 1) * P],
                                     ygt[:, tcc * TC:(tcc + 1) * TC],
                                     start=True, stop=True)
                    nc.scalar.copy(osb[:, tcc * TC:(tcc + 1) * TC], ps[:])
                nc.sync.dma_start(out_d[dt, ot * P:(ot + 1) * P, :], osb[:])

            if True:
                for dt in range(NDT):
                    yps = ypp.tile([P, L], F32, tag="yps", name=f"yps{dt}")
                    # seed with +y0
                    for c in range(NTC):
                        nc.tensor.matmul(yps[:, c * TC:(c + 1) * TC], idm[:],
                                         y0[dt][:, c * TC:(c + 1) * TC],
                                         start=True, stop=False)
                    for n in range(D_STATE):
                        bb = bcast_full(1, n)
                        cb = bcast_full(1, D_STATE + n)
                        dA = dap.tile([P, L], F16, tag="da")
                        nc.scalar.activation(dA[:], delta[1, dt][:], AT.Exp,
                                             bias=0.0,
                                             scale=acol[1, dt][:, n:n + 1])
                        dBu = dap.tile([P, L], F16, tag="da")
                        pl.mult(L).tensor_tensor(dBu[:], v16[1, dt][:], bb[:],
                                                 OP.mult)
                        hh = hpool.tile([P, L], F16, tag="h")
                        pl.dve(L, scan=True).tensor_tensor_scan(
                            hh[:], dA[:], dBu[:], 0.0, OP.mult, OP.add)
                        g = gp.tile([P, L], F16, tag="g")
                        pl.mult(L).tensor_tensor(g[:], hh[:], cb[:], OP.mult)
                        # accumulate -g into psum
                        for c in range(NTC):
                            nc.tensor.matmul(yps[:, c * TC:(c + 1) * TC],
                                             nidm[:],
                                             g[:, c * TC:(c + 1) * TC],
                                             start=False,
                                             stop=(n == D_STATE - 1))
                        if pending and 4 <= n <= 11:
                            pdt, pygt = pending[0]
                            emit_out(pdt, pygt, n - 4)
                            if n == 11:
                                pending.pop(0)

                    # ydiff(+D-term) from psum, then gate
                    yd2 = gp.tile([P, L], F16, tag="g")
                    pl.dve(L)
                    nc.vector.scalar_tensor_tensor(yd2[:], u16[dt][:],
                                                   ddiff[dt][:, 0:1], yps[:],
                                                   OP.mult, OP.add)
                    sz = gp.tile([P, L], F16, tag="g")
                    nc.scalar.activation(sz[:], z16[dt][:], AT.Silu)
                    ygt = bigp.tile([P, L + 3], F16, tag=f"x16_{dt}")
                    pl.mult(L).tensor_tensor(ygt[:, 0:L], yd2[:], sz[:],
                                             OP.mult)
                    if dt == NDT - 1:
                        for ot in range(D_MODEL // P):
                            emit_out(dt, ygt, ot)
                    else:
                        pending.append((dt, ygt))

    nc.finalize()
    return nc


def _get_nc():
    if "nc" not in _CACHE:
        _CACHE["nc"] = _build()
    return _CACHE["nc"]


def kernel(hidden_states, in_proj_w, conv_w, conv_b,
           x1_proj_w, dt1_proj_w, dt1_proj_b, A1_log, D1,
           x2_proj_w, dt2_proj_w, dt2_proj_b, A2_log, D2,
           out_proj_w):
    import os
    from concourse.bass_utils import run_bass_kernel_spmd
    try:
        import antenv.axon_hooks  # noqa: F401
    except ImportError:
        os.environ["BASS_NEVER_TRACE"] = "1"

    f32 = np.float32
    f16 = np.float16
    hidden_states = np.asarray(hidden_states, f32)
    in_proj_w = np.asarray(in_proj_w, f32)
    conv_w = np.asarray(conv_w, f32)
    conv_b = np.asarray(conv_b, f32)
    out_proj_w = np.asarray(out_proj_w, f32)

    hT16 = np.ascontiguousarray(hidden_states[0].T).astype(f16)
    A1 = -np.exp(np.asarray(A1_log, f32))
    A2 = -np.exp(np.asarray(A2_log, f32))
    Dd = (np.asarray(D1, f32) - np.asarray(D2, f32))

    eye = np.eye(P, dtype=f16)
    idm = np.stack([eye, -eye])

    xp = [np.asarray(x1_proj_w, f32), np.asarray(x2_proj_w, f32)]
    dtpw = [np.asarray(dt1_proj_w, f32), np.asarray(dt2_proj_w, f32)]
    dtb = [np.asarray(dt1_proj_b, f32), np.asarray(dt2_proj_b, f32)]
    Acols = [A1, A2]

    in_maps = []
    for c in range(NCORES):
        ds = slice(c * DLOC, (c + 1) * DLOC)
        ipw_loc = np.concatenate([in_proj_w[ds], in_proj_w[D_INNER:][ds]], 0)
        in_maps.append({
            "hT": hT16,
            "ipwT": np.ascontiguousarray(ipw_loc.T).astype(f16),
            "convw": np.ascontiguousarray(conv_w[ds]).astype(f32),
            "convb": np.ascontiguousarray(conv_b[ds][:, None]).astype(f32),
            "xpwT": np.ascontiguousarray(
                np.concatenate([xp[0][:, ds], xp[1][:, ds]], 0).T).astype(f16),
            "dtpwT": np.ascontiguousarray(
                np.stack([dtpw[0][ds].T, dtpw[1][ds].T])).astype(f32),
            "dtb": np.ascontiguousarray(
                np.stack([dtb[0][ds][:, None], dtb[1][ds][:, None]])).astype(f32),
            "acol": np.ascontiguousarray(
                np.stack([Acols[0][ds], Acols[1][ds]])).astype(f32),
            "ddiff": np.ascontiguousarray(Dd[ds][:, None]).astype(f32),
            "opwT": np.ascontiguousarray(out_proj_w[:, ds].T).astype(f16),
            "idm": idm,
        })

    nc = _get_nc()
    res = run_bass_kernel_spmd(nc, in_maps, core_ids=list(range(NCORES)))
    _CACHE["last_res"] = res
    out = np.zeros((D_MODEL, L), f32)
    for r in res.results:
        out += r["outp"].astype(f32).sum(axis=0)
    return np.ascontiguousarray(out.T)[None].astype(f32)
